# revision 1
# baseline (speedup 1.0000x reference)
"""Complex multihead attention (split softmax) on 8 Trainium2 NeuronCores.

Sharding: data-parallel over batch (B=4) x tensor-parallel over heads
(16 heads -> 2 groups of 8). core = b*2 + head_group. Each core computes
Q/K/V projections for its 8 heads, per-head attention, and a partial O
projection over its heads' columns; partials are summed on the host.

Device math notes (all validated against the reference in fp64/fp32):
 - K bias dropped: adds a per-query constant to logits -> softmax invariant.
 - V bias folded to a host-side constant: softmax rows sum to 1, so the
   bias contributes Wo @ ((1+i)*bv) to every output position.
 - Q bias applied during PSUM->SBUF evacuation (per-partition ACT bias).
 - All matmuls run as float32r (full-rate fp32 PE path).
 - Scores are computed transposed (St[sk, sq]) so softmax reduction over
   keys becomes a partition reduction done by ones-vector matmuls; the
   1/sum normalizer is broadcast across partitions with a K=1 matmul.
"""

import numpy as np

import concourse.bass as bass
from concourse import bacc
import concourse.mybir as mybir
import concourse.tile as tile
from concourse.bass_utils import run_bass_kernel_spmd

S, B, E, H, D = 1024, 4, 1024, 16, 64
HPC = 8            # heads per core
EH = HPC * D       # 512
N_CORES = 8
F32 = mybir.dt.float32
F32R = mybir.dt.float32r
AF = mybir.ActivationFunctionType

_NC_CACHE = []


def _r(ap):
    return ap


def _emit(tc):
    nc = tc.nc
    xq = nc.dram_tensor("xq", [2 * E, S], F32R, kind="ExternalInput").ap()
    xk = nc.dram_tensor("xk", [2 * E, S], F32R, kind="ExternalInput").ap()
    xv = nc.dram_tensor("xv", [2 * E, S], F32R, kind="ExternalInput").ap()
    wq = nc.dram_tensor("wq", [2 * E, 2 * EH], F32R, kind="ExternalInput").ap()
    wk = nc.dram_tensor("wk", [2 * E, 2 * EH], F32R, kind="ExternalInput").ap()
    wv = nc.dram_tensor("wv", [2 * E, 2 * EH], F32R, kind="ExternalInput").ap()
    wor = nc.dram_tensor("wor", [2 * EH, E], F32R, kind="ExternalInput").ap()
    woi = nc.dram_tensor("woi", [2 * EH, E], F32R, kind="ExternalInput").ap()
    bq = nc.dram_tensor("bq", [128, HPC], F32, kind="ExternalInput").ap()
    onesd = nc.dram_tensor("onesd", [128, 128], F32R, kind="ExternalInput").ap()
    ytr = nc.dram_tensor("ytr", [E, S], F32, kind="ExternalOutput").ap()
    yti = nc.dram_tensor("yti", [E, S], F32, kind="ExternalOutput").ap()

    store = tc.alloc_tile_pool(name="store", bufs=1)
    qcat = store.tile([128, HPC * S], F32R)   # per head j: [Qr;Qi]^T at cols j*S..
    kcat = store.tile([128, HPC * S], F32R)
    vnat = store.tile([128, HPC * S], F32R)   # per s-tile t: cols t*1024+(j,ri,d)
    bq_sb = store.tile([128, HPC], F32)
    ones_col = store.tile([128, 1], F32R)
    ones_sq = store.tile([128, 128], F32R)
    nc.sync.dma_start(out=bq_sb, in_=bq)
    nc.sync.dma_start(out=ones_col, in_=onesd[:, 0:1])
    nc.sync.dma_start(out=ones_sq, in_=onesd)

    # ---------------- projections ----------------
    with tc.tile_pool(name="xp", bufs=20) as xp, \
         tc.tile_pool(name="wp", bufs=4) as wp, \
         tc.tile_pool(name="pp", bufs=8, space="PSUM") as pp:

        for which, xdram, wdram, dest, bias in (
            ("q", xq, wq, qcat, bq_sb),
            ("k", xk, wk, kcat, None),
        ):
            xs = []
            w0 = []
            for k in range(16):
                wt = wp.tile([128, 512], F32R, tag="w", name=f"w{which}0{k}")
                nc.sync.dma_start(
                    out=wt, in_=wdram[k * 128:(k + 1) * 128, 0:512])
                w0.append(wt)
                xt = xp.tile([128, S], F32R, tag="x", name=f"x{which}{k}")
                nc.sync.dma_start(out=xt, in_=xdram[k * 128:(k + 1) * 128, :])
                xs.append(xt)
            for grp in range(2):
                if grp == 0:
                    wts = w0
                else:
                    wts = []
                    for k in range(16):
                        wt = wp.tile([128, 512], F32R, tag="w",
                                     name=f"w{which}1{k}")
                        nc.sync.dma_start(
                            out=wt,
                            in_=wdram[k * 128:(k + 1) * 128, 512:1024])
                        wts.append(wt)
                ps = [[pp.tile([128, 512], F32, tag="pp", name=f"p{which}{grp}{j}{hf}")
                       for hf in range(2)] for j in range(4)]
                for k in range(16):
                    for j in range(4):
                        lhsT = _r(wts[k][:, j * 128:(j + 1) * 128])
                        for hf in range(2):
                            nc.tensor.matmul(
                                ps[j][hf], lhsT,
                                _r(xs[k][:, hf * 512:(hf + 1) * 512]),
                                start=(k == 0), stop=(k == 15))
                for j in range(4):
                    h = grp * 4 + j
                    for hf in range(2):
                        dst = dest[:, h * S + hf * 512: h * S + (hf + 1) * 512]
                        if bias is not None:
                            nc.scalar.activation(dst, ps[j][hf], AF.Identity,
                                                 bias=bias[:, h:h + 1])
                        else:
                            nc.vector.tensor_copy(dst, ps[j][hf])

        # V projection, natural layout: out[s, (j, r/i, d)]
        xs = []
        wv0 = []
        for k in range(16):
            wt = wp.tile([128, 512], F32R, tag="w", name=f"wv0{k}")
            nc.sync.dma_start(out=wt, in_=wv[k * 128:(k + 1) * 128, 0:512])
            wv0.append(wt)
            xt = xp.tile([128, S], F32R, tag="x", name=f"xv{k}")
            nc.sync.dma_start(out=xt, in_=xv[k * 128:(k + 1) * 128, :])
            xs.append(xt)
        for hf in range(2):
            if hf == 0:
                wts = wv0
            else:
                wts = []
                for k in range(16):
                    wt = wp.tile([128, 512], F32R, tag="w", name=f"wv1{k}")
                    nc.sync.dma_start(
                        out=wt, in_=wv[k * 128:(k + 1) * 128, 512:1024])
                    wts.append(wt)
            ps = [pp.tile([128, 512], F32, tag="pp", name=f"pv{hf}{st}")
                  for st in range(8)]
            for k in range(16):
                for st in range(8):
                    nc.tensor.matmul(ps[st],
                                     _r(xs[k][:, st * 128:(st + 1) * 128]),
                                     _r(wts[k][:, :]),
                                     start=(k == 0), stop=(k == 15))
            for st in range(8):
                nc.vector.tensor_copy(
                    vnat[:, st * 1024 + hf * 512: st * 1024 + (hf + 1) * 512],
                    ps[st])

    # ---------------- attention ----------------
    attn_pool = tc.alloc_tile_pool(name="attnp", bufs=1)
    attn = attn_pool.tile([128, HPC * S], F32R)  # per head j: [or;oi]^T

    # O-projection weights prefetch pool (consumed in the next phase); the
    # gathered DMAs have no upstream deps so they fill DMA idle time here.
    wop = tc.alloc_tile_pool(name="wop", bufs=4)
    wo_tiles = {}
    for part, wo_d in ((0, wor), (1, woi)):
        wo_re = wo_d.rearrange("(j p) n -> p j n", p=128)  # [128, 8, 1024]
        for m in range(8):
            wt = wop.tile([128, 8, 128], F32R, tag="wo", name=f"wo{part}{m}")
            nc.sync.dma_start(out=wt, in_=wo_re[:, :, m * 128:(m + 1) * 128])
            wo_tiles[(part, m)] = wt

    with tc.tile_pool(name="asb", bufs=2) as asb, \
         tc.tile_pool(name="stp", bufs=2, space="PSUM") as stp, \
         tc.tile_pool(name="opp", bufs=2, space="PSUM") as opp, \
         tc.tile_pool(name="smp", bufs=4, space="PSUM") as smp:

        def finalize(fin):
            """Normalize + combine for a finished (j, qi) iteration. Deferred
            one iteration so the PE queue never stalls on the reciprocal."""
            j, qi, s_r, s_i, o1, o2 = fin
            sq0 = qi * 512
            rcp_r = asb.tile([128, 512], F32, tag="rcpf", name=f"rf{j}{qi}")
            rcp_i = asb.tile([128, 512], F32, tag="rcpc", name=f"rc{j}{qi}")
            nc.vector.reciprocal_approx_fast(rcp_r[0:64, :], s_r[0:64, :])
            nc.vector.reciprocal_approx_fast(rcp_i[0:64, :], s_i[0:64, :])
            # replicate to the upper partition half via SBUF-to-SBUF DMA
            nc.sync.dma_start(out=rcp_r[64:128, :], in_=rcp_r[0:64, :])
            nc.sync.dma_start(out=rcp_i[64:128, :], in_=rcp_i[0:64, :])
            t1 = asb.tile([128, 512], F32, tag="t1", name=f"t1{j}{qi}")
            t2 = asb.tile([128, 512], F32, tag="t1", name=f"t2{j}{qi}")
            dst = attn[:, j * S + sq0: j * S + sq0 + 512]
            # real rows: (PrVr)^T/sum_r - (PiVi)^T/sum_i
            nc.vector.tensor_mul(t1[0:64, :], o1[0:64, :], rcp_r[0:64, :])
            nc.vector.tensor_mul(t2[0:64, :], o2[0:64, :], rcp_i[0:64, :])
            nc.vector.tensor_sub(dst[0:64, :], t1[0:64, :], t2[0:64, :])
            # imag rows: (PrVi)^T/sum_r + (PiVr)^T/sum_i
            nc.vector.tensor_mul(t1[64:128, :], o1[64:128, :], rcp_r[64:128, :])
            nc.vector.tensor_mul(t2[64:128, :], o2[64:128, :], rcp_i[64:128, :])
            nc.vector.tensor_add(dst[64:128, :], t1[64:128, :], t2[64:128, :])

        pending = None
        for j in range(HPC):
            qh = qcat[:, j * S:(j + 1) * S]
            qv1 = asb.tile([128, S], F32R, tag="qv1", name=f"qv1_{j}")
            qv2 = asb.tile([128, S], F32R, tag="qv2", name=f"qv2_{j}")
            nc.vector.tensor_copy(qv1[0:64, :], qh[0:64, :])
            nc.vector.tensor_scalar_mul(qv1[64:128, :], qh[64:128, :], -1.0)
            # partition swap -> SBUF-to-SBUF DMA
            nc.sync.dma_start(out=qv2[0:64, :], in_=qh[64:128, :])
            nc.sync.dma_start(out=qv2[64:128, :], in_=qh[0:64, :])
            # per-head [Vi|Vr] swapped copy for the o2 products
            vsw = asb.tile([128, S], F32R, tag="vsw", name=f"vsw_{j}")
            for t in range(8):
                base = t * 1024 + j * 128
                nc.vector.tensor_copy(vsw[:, t * 128: t * 128 + 64],
                                      vnat[:, base + 64: base + 128])
                nc.vector.tensor_copy(vsw[:, t * 128 + 64: t * 128 + 128],
                                      vnat[:, base: base + 64])

            for qi in range(2):
                sq0 = qi * 512
                s_r = smp.tile([128, 512], F32, tag="sums", name=f"sr_{j}{qi}")
                s_i = smp.tile([128, 512], F32, tag="sums", name=f"si_{j}{qi}")
                o1 = opp.tile([128, 512], F32, tag="opv", name=f"o1_{j}{qi}")
                o2 = opp.tile([128, 512], F32, tag="opv", name=f"o2_{j}{qi}")
                for t in range(8):
                    st_r = stp.tile([128, 512], F32, tag="st", name=f"str{j}{qi}{t}")
                    st_i = stp.tile([128, 512], F32, tag="st", name=f"sti{j}{qi}{t}")
                    kl = kcat[:, j * S + t * 128: j * S + (t + 1) * 128]
                    nc.tensor.matmul(st_r, kl, qv1[:, sq0:sq0 + 512],
                                     start=True, stop=True)
                    nc.tensor.matmul(st_i, kl, qv2[:, sq0:sq0 + 512],
                                     start=True, stop=True)
                    pt_r = asb.tile([128, 512], F32R, tag="pt", bufs=3,
                                    name=f"ptr{j}{qi}{t}")
                    pt_i = asb.tile([128, 512], F32R, tag="pt", bufs=3,
                                    name=f"pti{j}{qi}{t}")
                    nc.scalar.activation(pt_r, st_r, AF.Exp, scale=0.125)
                    nc.scalar.activation(pt_i, st_i, AF.Exp, scale=0.125)
                    nc.tensor.matmul(s_r[0:64, :], ones_sq[:, 0:64], pt_r,
                                     start=(t == 0), stop=(t == 7))
                    nc.tensor.matmul(s_i[0:64, :], ones_sq[:, 0:64], pt_i,
                                     start=(t == 0), stop=(t == 7))
                    vl = vnat[:, t * 1024 + j * 128: t * 1024 + (j + 1) * 128]
                    nc.tensor.matmul(o1, vl, pt_r,
                                     start=(t == 0), stop=(t == 7))
                    nc.tensor.matmul(o2, vsw[:, t * 128:(t + 1) * 128], pt_i,
                                     start=(t == 0), stop=(t == 7))
                if pending is not None:
                    finalize(pending)
                pending = (j, qi, s_r, s_i, o1, o2)
        finalize(pending)

    # ---------------- O projection (partials) ----------------
    with tc.tile_pool(name="ytp", bufs=4) as ytp, \
         tc.tile_pool(name="pop", bufs=4, space="PSUM") as pop:
        for part, wo_d, yt_d in ((0, wor, ytr), (1, woi, yti)):
            for m in range(8):
                wt = wo_tiles[(part, m)]
                for hf in range(2):
                    ps = pop.tile([128, 512], F32, tag="po", name=f"po{part}{m}{hf}")
                    for jj in range(8):
                        nc.tensor.matmul(
                            ps, _r(wt[:, jj, :]),
                            _r(attn[:, jj * S + hf * 512: jj * S + (hf + 1) * 512]),
                            start=(jj == 0), stop=(jj == 7))
                    yt_t = ytp.tile([128, 512], F32, tag="yt", name=f"yt{part}{m}{hf}")
                    nc.vector.tensor_copy(yt_t, ps)
                    nc.sync.dma_start(
                        out=yt_d[m * 128:(m + 1) * 128, hf * 512:(hf + 1) * 512],
                        in_=yt_t)

    wop.release()
    attn_pool.release()
    store.release()


def build_module():
    nc = bacc.Bacc("TRN2", target_bir_lowering=False)
    with tile.TileContext(nc) as tc:
        _emit(tc)
    nc.compile()
    return nc


def _get_nc():
    if not _NC_CACHE:
        _NC_CACHE.append(build_module())
    return _NC_CACHE[0]


def prep_core(inp, core):
    """Host-side shard prep for one core."""
    b, hg = divmod(core, 2)
    hs, he = hg * EH, (hg + 1) * EH

    def xcat(xr, xi):
        return np.ascontiguousarray(
            np.concatenate([xr[:, b, :].T, xi[:, b, :].T], axis=0), dtype=np.float32)

    def w_prep(wr, wi):
        A = wr[hs:he, :].T
        Bm = wi[hs:he, :].T
        top = np.concatenate([A.reshape(E, HPC, D), Bm.reshape(E, HPC, D)], axis=2)
        bot = np.concatenate([-Bm.reshape(E, HPC, D), A.reshape(E, HPC, D)], axis=2)
        return np.ascontiguousarray(
            np.concatenate([top.reshape(E, 2 * EH), bot.reshape(E, 2 * EH)], axis=0),
            dtype=np.float32)

    def wo_prep(w_top, w_bot):
        Ct = w_top[:, hs:he].T.reshape(HPC, D, E)
        Dt = w_bot[:, hs:he].T.reshape(HPC, D, E)
        return np.ascontiguousarray(
            np.concatenate([Ct, Dt], axis=1).reshape(2 * EH, E), dtype=np.float32)

    bqp = np.empty((128, HPC), np.float32)
    for j in range(HPC):
        h = hg * HPC + j
        bqp[:64, j] = inp["bq_r"][h * D:(h + 1) * D]
        bqp[64:, j] = inp["bq_i"][h * D:(h + 1) * D]

    return dict(
        xq=xcat(inp["query_r"], inp["query_i"]),
        xk=xcat(inp["key_r"], inp["key_i"]),
        xv=xcat(inp["value_r"], inp["value_i"]),
        wq=w_prep(inp["wq_r"], inp["wq_i"]),
        wk=w_prep(inp["wk_r"], inp["wk_i"]),
        wv=w_prep(inp["wv_r"], inp["wv_i"]),
        wor=wo_prep(inp["wo_r"], -inp["wo_i"]),
        woi=wo_prep(inp["wo_i"], inp["wo_r"]),
        bq=bqp,
        onesd=np.ones((128, 128), np.float32),
    )


def host_combine(results, inp):
    """Sum per-core partials, add the host-side constant, untranspose."""
    bvr = inp["bv_r"].astype(np.float64)
    bvi = inp["bv_i"].astype(np.float64)
    wr = inp["wo_r"].astype(np.float64)
    wi = inp["wo_i"].astype(np.float64)
    vb_r = bvr - bvi
    vb_i = bvr + bvi
    yc_r = (wr @ vb_r - wi @ vb_i + inp["bo_r"]).astype(np.float32)
    yc_i = (wr @ vb_i + wi @ vb_r + inp["bo_i"]).astype(np.float32)

    out = np.empty((S, B, E, 2), np.float32)
    for b in range(B):
        yr = results[2 * b]["ytr"] + results[2 * b + 1]["ytr"]
        yi = results[2 * b]["yti"] + results[2 * b + 1]["yti"]
        out[:, b, :, 0] = yr.T + yc_r
        out[:, b, :, 1] = yi.T + yc_i
    return out


def kernel(**inputs):
    inputs = {k: np.asarray(v) for k, v in inputs.items()}
    nc = _get_nc()
    in_maps = [prep_core(inputs, c) for c in range(N_CORES)]
    res = run_bass_kernel_spmd(nc, in_maps, core_ids=list(range(N_CORES)))
    return host_combine(res.results, inputs)



# revision 2
# speedup vs baseline: 1.3413x; 1.3413x over previous
"""Complex multihead attention (split softmax) on 8 Trainium2 NeuronCores.

Sharding: data-parallel over batch (B=4) x tensor-parallel over heads
(16 heads -> 2 groups of 8). core = b*2 + head_group. Each core computes
Q/K/V projections for its 8 heads, per-head attention, and a partial O
projection over its heads' columns; partials are summed on the host.

Device math notes (validated against the reference):
 - K bias dropped (softmax invariant), V bias folded to a host constant,
   Q bias applied during PSUM->SBUF evacuation (per-partition ACT bias).
 - All matmul operands are bf16 (PSUM accumulates fp32); outputs fp32.
 - wq/bq carry a sign flip on the Qi half so qcat == [Qr; -Qi] == qv1.
 - Scores are computed transposed (St[sk, sq]); st_r and st_i share one
   2-bank PSUM tile so a single ACT Exp covers both.
 - Softmax denominators: DVE pairwise adds of exp tiles (8->4), then
   4+4 ones-matmuls accumulate in PSUM; reciprocal broadcast to all 128
   partitions via M=128 ones lhsT.
 - The o1/o2 AV matmuls lag the st matmuls by one t-step so the PE never
   waits on the ACT exp round-trip.
 - V is evacuated twice: natural layout and [Vi|Vr]-half-swapped, which
   feeds the o2 products without any per-head shuffling.
"""

import numpy as np
import ml_dtypes

import concourse.bass as bass
from concourse import bacc
import concourse.mybir as mybir
import concourse.tile as tile
from concourse.bass_utils import run_bass_kernel_spmd

S, B, E, H, D = 1024, 4, 1024, 16, 64
HPC = 8            # heads per core
EH = HPC * D       # 512
N_CORES = 8
F32 = mybir.dt.float32
BF16 = mybir.dt.bfloat16
AF = mybir.ActivationFunctionType
BFNP = ml_dtypes.bfloat16

_NC_CACHE = []


def _emit(tc):
    nc = tc.nc
    xq = nc.dram_tensor("xq", [2 * E, S], BF16, kind="ExternalInput").ap()
    xk = nc.dram_tensor("xk", [2 * E, S], BF16, kind="ExternalInput").ap()
    xv = nc.dram_tensor("xv", [2 * E, S], BF16, kind="ExternalInput").ap()
    wq = nc.dram_tensor("wq", [2 * E, 2 * EH], BF16, kind="ExternalInput").ap()
    wk = nc.dram_tensor("wk", [2 * E, 2 * EH], BF16, kind="ExternalInput").ap()
    wv = nc.dram_tensor("wv", [2 * E, 2 * EH], BF16, kind="ExternalInput").ap()
    # wo: [part(2)][p(128)][m(8)][j(8)][dcol(128)] pre-arranged on host
    wo = nc.dram_tensor("wo", [2, 128, 8, 8, 128], BF16,
                        kind="ExternalInput").ap()
    bq = nc.dram_tensor("bq", [128, HPC], F32, kind="ExternalInput").ap()
    onesd = nc.dram_tensor("onesd", [128, 128], BF16, kind="ExternalInput").ap()
    ytr = nc.dram_tensor("ytr", [E, S], F32, kind="ExternalOutput").ap()
    yti = nc.dram_tensor("yti", [E, S], F32, kind="ExternalOutput").ap()

    store = tc.alloc_tile_pool(name="store", bufs=1)
    qcat = store.tile([128, HPC, S], BF16)       # [Qr; -Qi] per head
    kcat = store.tile([128, HPC, S], BF16)       # [Kr; Ki] per head
    vnat = store.tile([128, 8, HPC, 2, 64], BF16)  # (st, j, ri, d)
    vnsw = store.tile([128, 8, HPC, 2, 64], BF16)  # ri swapped: [Vi|Vr]
    attn = store.tile([128, HPC, S], BF16)       # [or; oi] per head
    bq_sb = store.tile([128, HPC], F32)
    ones_sb = store.tile([128, 128], BF16)
    nc.sync.dma_start(out=bq_sb, in_=bq)
    nc.sync.dma_start(out=ones_sb, in_=onesd)

    # ---------------- projections ----------------
    with tc.tile_pool(name="xp", bufs=18) as xp, \
         tc.tile_pool(name="wp", bufs=4) as wp, \
         tc.tile_pool(name="pp", bufs=8, space="PSUM") as pp:

        for which, xdram, wdram, dest, bias in (
            ("q", xq, wq, qcat, bq_sb),
            ("k", xk, wk, kcat, None),
        ):
            xs = []
            w0 = []
            for k in range(16):
                wt = wp.tile([128, 512], BF16, tag="w", name=f"w{which}0{k}")
                nc.sync.dma_start(
                    out=wt, in_=wdram[k * 128:(k + 1) * 128, 0:512])
                w0.append(wt)
                xt = xp.tile([128, S], BF16, tag="x", name=f"x{which}{k}")
                nc.sync.dma_start(out=xt, in_=xdram[k * 128:(k + 1) * 128, :])
                xs.append(xt)
            for grp in range(2):
                if grp == 0:
                    wts = w0
                else:
                    wts = []
                    for k in range(16):
                        wt = wp.tile([128, 512], BF16, tag="w",
                                     name=f"w{which}1{k}")
                        nc.sync.dma_start(
                            out=wt,
                            in_=wdram[k * 128:(k + 1) * 128, 512:1024])
                        wts.append(wt)
                ps = [[pp.tile([128, 512], F32, tag="pp",
                               name=f"p{which}{grp}{j}{hf}")
                       for hf in range(2)] for j in range(4)]
                for k in range(16):
                    for j in range(4):
                        lhsT = wts[k][:, j * 128:(j + 1) * 128]
                        for hf in range(2):
                            nc.tensor.matmul(
                                ps[j][hf], lhsT,
                                xs[k][:, hf * 512:(hf + 1) * 512],
                                start=(k == 0), stop=(k == 15))
                for j in range(4):
                    h = grp * 4 + j
                    for hf in range(2):
                        dst = dest[:, h, hf * 512:(hf + 1) * 512]
                        if bias is not None:
                            nc.scalar.activation(dst, ps[j][hf], AF.Identity,
                                                 bias=bias[:, h:h + 1])
                        else:
                            nc.scalar.activation(dst, ps[j][hf], AF.Copy)

        # V projection, natural layout: psum partitions = tokens
        xs = []
        wv0 = []
        for k in range(16):
            wt = wp.tile([128, 512], BF16, tag="w", name=f"wv0{k}")
            nc.sync.dma_start(out=wt, in_=wv[k * 128:(k + 1) * 128, 0:512])
            wv0.append(wt)
            xt = xp.tile([128, S], BF16, tag="x", name=f"xv{k}")
            nc.sync.dma_start(out=xt, in_=xv[k * 128:(k + 1) * 128, :])
            xs.append(xt)
        for hf in range(2):
            if hf == 0:
                wts = wv0
            else:
                wts = []
                for k in range(16):
                    wt = wp.tile([128, 512], BF16, tag="w", name=f"wv1{k}")
                    nc.sync.dma_start(
                        out=wt, in_=wv[k * 128:(k + 1) * 128, 512:1024])
                    wts.append(wt)
            ps = [pp.tile([128, 4, 2, 64], F32, tag="pp", name=f"pv{hf}{st}")
                  for st in range(8)]
            for k in range(16):
                for st in range(8):
                    nc.tensor.matmul(ps[st],
                                     xs[k][:, st * 128:(st + 1) * 128],
                                     wts[k][:, :],
                                     start=(k == 0), stop=(k == 15))
            for st in range(8):
                jl = slice(hf * 4, hf * 4 + 4)
                nc.scalar.activation(vnat[:, st, jl, :, :], ps[st], AF.Copy)
                nc.scalar.activation(vnsw[:, st, jl, 1, :],
                                     ps[st][:, :, 0, :], AF.Copy)
                nc.scalar.activation(vnsw[:, st, jl, 0, :],
                                     ps[st][:, :, 1, :], AF.Copy)

    # O-projection weight prefetch: no upstream deps, fills DMA idle time.
    wop = tc.alloc_tile_pool(name="wop", bufs=1)
    wo_tiles = {}
    for part in range(2):
        for m in range(8):
            wt = wop.tile([128, 8, 128], BF16, name=f"wo{part}{m}")
            nc.sync.dma_start(out=wt, in_=wo[part, :, m, :, :])
            wo_tiles[(part, m)] = wt

    # ---------------- attention ----------------
    with tc.tile_pool(name="asb", bufs=2) as asb, \
         tc.tile_pool(name="ptp", bufs=2) as ptp, \
         tc.tile_pool(name="stp", bufs=2, space="PSUM") as stp, \
         tc.tile_pool(name="opp", bufs=2, space="PSUM") as opp, \
         tc.tile_pool(name="smp", bufs=2, space="PSUM") as smp:

        def finalize(fin):
            """Normalize + combine a finished (j, qi); deferred one iteration
            so PE/ACT never stall on the reciprocal chain."""
            j, qi, o1s, o2s, s_r, s_i = fin
            sq0 = qi * 512
            rcp_r = asb.tile([128, 512], F32, tag="rcp", name=f"rr{j}{qi}")
            rcp_i = asb.tile([128, 512], F32, tag="rcp", name=f"rc{j}{qi}")
            nc.vector.reciprocal_approx_fast(rcp_r, s_r)
            nc.vector.reciprocal_approx_fast(rcp_i, s_i)
            t1 = asb.tile([128, 512], BF16, tag="t12", name=f"t1{j}{qi}")
            t2 = asb.tile([128, 512], BF16, tag="t12", name=f"t2{j}{qi}")
            nc.gpsimd.tensor_mul(t1, o1s, rcp_r)
            nc.gpsimd.tensor_mul(t2, o2s, rcp_i)
            dst = attn[:, j, sq0:sq0 + 512]
            nc.vector.tensor_sub(dst[0:64, :], t1[0:64, :], t2[0:64, :])
            nc.vector.tensor_add(dst[64:128, :], t1[64:128, :], t2[64:128, :])

        pending = None
        for j in range(HPC):
            # qv2 = [Qi; Qr]: DMA partition swap + DVE negate (Qi = -qcat bot)
            qv2 = asb.tile([128, S], BF16, tag="qv2", name=f"qv2_{j}")
            qsw = asb.tile([64, S], BF16, tag="qsw", name=f"qsw_{j}")
            nc.sync.dma_start(out=qsw[0:64, :], in_=qcat[64:128, j, :])
            nc.sync.dma_start(out=qv2[64:128, :], in_=qcat[0:64, j, :])
            nc.vector.tensor_scalar_mul(qv2[0:64, :], qsw[0:64, :], -1.0)

            for qi in range(2):
                sq0 = qi * 512
                qv1s = qcat[:, j, sq0:sq0 + 512]
                qv2s = qv2[:, sq0:sq0 + 512]
                o1 = opp.tile([128, 512], F32, tag="o", name=f"o1_{j}{qi}")
                o2 = opp.tile([128, 512], F32, tag="o", name=f"o2_{j}{qi}")
                pt = ptp.tile([128, 8, 2, 512], BF16, tag="pt",
                              name=f"pt{j}{qi}")
                sts = [None] * 8
                prs = []
                for t in range(8):
                    st = stp.tile([128, 2, 512], F32, tag="st",
                                  name=f"st{j}{qi}{t}")
                    sts[t] = st
                    kl = kcat[:, j, t * 128:(t + 1) * 128]
                    nc.tensor.matmul(st[:, 0, :], kl, qv1s,
                                     start=True, stop=True)
                    nc.tensor.matmul(st[:, 1, :], kl, qv2s,
                                     start=True, stop=True)
                    nc.scalar.activation(pt[:, t, :, :], st, AF.Exp,
                                         scale=0.125)
                    if t >= 1:
                        u = t - 1
                        nc.tensor.matmul(o1, vnat[:, u, j, :, :],
                                         pt[:, u, 0, :],
                                         start=(u == 0), stop=False)
                        nc.tensor.matmul(o2, vnsw[:, u, j, :, :],
                                         pt[:, u, 1, :],
                                         start=(u == 0), stop=False)
                    if t % 2 == 1:
                        pr = asb.tile([128, 2, 512], BF16, tag="pr",
                                      bufs=4, name=f"pr{j}{qi}{t}")
                        nc.vector.tensor_add(pr, pt[:, t - 1, :, :],
                                             pt[:, t, :, :])
                        prs.append(pr)
                    if t == 2 and pending is not None:
                        finalize(pending)
                        pending = None
                nc.tensor.matmul(o1, vnat[:, 7, j, :, :], pt[:, 7, 0, :],
                                 start=False, stop=True)
                nc.tensor.matmul(o2, vnsw[:, 7, j, :, :], pt[:, 7, 1, :],
                                 start=False, stop=True)
                # evacuate o1/o2 to SBUF (bf16) so the PSUM banks recycle
                o1s = asb.tile([128, 512], BF16, tag="osb", bufs=4,
                               name=f"o1s{j}{qi}")
                o2s = asb.tile([128, 512], BF16, tag="osb", bufs=4,
                               name=f"o2s{j}{qi}")
                nc.vector.tensor_copy(o1s, o1)
                nc.vector.tensor_copy(o2s, o2)
                # softmax denominators: 4 pair tiles -> 4+4 ones-matmuls
                s_r = smp.tile([128, 512], F32, tag="s", name=f"sr{j}{qi}")
                s_i = smp.tile([128, 512], F32, tag="s", name=f"si{j}{qi}")
                for pi, pr in enumerate(prs):
                    nc.tensor.matmul(s_r, ones_sb, pr[:, 0, :],
                                     start=(pi == 0), stop=(pi == 3))
                    nc.tensor.matmul(s_i, ones_sb, pr[:, 1, :],
                                     start=(pi == 0), stop=(pi == 3))
                pending = (j, qi, o1s, o2s, s_r, s_i)
        finalize(pending)

    # ---------------- O projection (partials) ----------------
    with tc.tile_pool(name="ytp", bufs=4) as ytp, \
         tc.tile_pool(name="pop", bufs=4, space="PSUM") as pop:
        for part, yt_d in ((0, ytr), (1, yti)):
            for m in range(8):
                wt = wo_tiles[(part, m)]
                for hf in range(2):
                    ps = pop.tile([128, 512], F32, tag="po",
                                  name=f"po{part}{m}{hf}")
                    for jj in range(8):
                        nc.tensor.matmul(
                            ps, wt[:, jj, :],
                            attn[:, jj, hf * 512:(hf + 1) * 512],
                            start=(jj == 0), stop=(jj == 7))
                    yt_t = ytp.tile([128, 512], F32, tag="yt",
                                    name=f"yt{part}{m}{hf}")
                    nc.scalar.activation(yt_t, ps, AF.Copy)
                    nc.sync.dma_start(
                        out=yt_d[m * 128:(m + 1) * 128,
                                 hf * 512:(hf + 1) * 512],
                        in_=yt_t)

    wop.release()
    store.release()


def build_module():
    nc = bacc.Bacc("TRN2", target_bir_lowering=False)
    with tile.TileContext(nc) as tc:
        _emit(tc)
    nc.compile()
    return nc


def _get_nc():
    if not _NC_CACHE:
        _NC_CACHE.append(build_module())
    return _NC_CACHE[0]


def prep_core(inp, core):
    """Host-side shard prep for one core."""
    b, hg = divmod(core, 2)
    hs, he = hg * EH, (hg + 1) * EH

    def xcat(xr, xi):
        return np.ascontiguousarray(
            np.concatenate([xr[:, b, :].T, xi[:, b, :].T], axis=0)
        ).astype(BFNP)

    def w_prep(wr, wi, flip):
        A = wr[hs:he, :].T
        Bm = wi[hs:he, :].T
        top = np.concatenate([A.reshape(E, HPC, D), Bm.reshape(E, HPC, D)],
                             axis=2)
        bot = np.concatenate([-Bm.reshape(E, HPC, D), A.reshape(E, HPC, D)],
                             axis=2)
        W = np.concatenate([top.reshape(E, 2 * EH), bot.reshape(E, 2 * EH)],
                           axis=0)
        if flip:
            W = W.reshape(2 * E, HPC, 2, D).copy()
            W[:, :, 1, :] *= -1.0
            W = W.reshape(2 * E, 2 * EH)
        return np.ascontiguousarray(W).astype(BFNP)

    def wo_prep(w_top, w_bot):
        Ct = w_top[:, hs:he].T.reshape(HPC, D, E)
        Dt = w_bot[:, hs:he].T.reshape(HPC, D, E)
        arr = np.concatenate([Ct, Dt], axis=1).reshape(2 * EH, E)
        A4 = arr.reshape(HPC, 128, 8, 128)          # (j, p, m, c)
        return np.ascontiguousarray(np.transpose(A4, (1, 2, 0, 3)))

    wo_both = np.stack([
        wo_prep(inp["wo_r"], -inp["wo_i"]),
        wo_prep(inp["wo_i"], inp["wo_r"]),
    ], axis=0).astype(BFNP)

    bqp = np.empty((128, HPC), np.float32)
    for j in range(HPC):
        h = hg * HPC + j
        bqp[:64, j] = inp["bq_r"][h * D:(h + 1) * D]
        bqp[64:, j] = -inp["bq_i"][h * D:(h + 1) * D]

    return dict(
        xq=xcat(inp["query_r"], inp["query_i"]),
        xk=xcat(inp["key_r"], inp["key_i"]),
        xv=xcat(inp["value_r"], inp["value_i"]),
        wq=w_prep(inp["wq_r"], inp["wq_i"], True),
        wk=w_prep(inp["wk_r"], inp["wk_i"], False),
        wv=w_prep(inp["wv_r"], inp["wv_i"], False),
        wo=wo_both,
        bq=bqp,
        onesd=np.ones((128, 128), BFNP),
    )


def host_combine(results, inp):
    """Sum per-core partials, add the host-side constant, untranspose."""
    bvr = inp["bv_r"].astype(np.float64)
    bvi = inp["bv_i"].astype(np.float64)
    wr = inp["wo_r"].astype(np.float64)
    wi = inp["wo_i"].astype(np.float64)
    vb_r = bvr - bvi
    vb_i = bvr + bvi
    yc_r = (wr @ vb_r - wi @ vb_i + inp["bo_r"]).astype(np.float32)
    yc_i = (wr @ vb_i + wi @ vb_r + inp["bo_i"]).astype(np.float32)

    out = np.empty((S, B, E, 2), np.float32)
    for b in range(B):
        yr = results[2 * b]["ytr"] + results[2 * b + 1]["ytr"]
        yi = results[2 * b]["yti"] + results[2 * b + 1]["yti"]
        out[:, b, :, 0] = yr.T + yc_r
        out[:, b, :, 1] = yi.T + yc_i
    return out


def kernel(**inputs):
    inputs = {k: np.asarray(v) for k, v in inputs.items()}
    nc = _get_nc()
    in_maps = [prep_core(inputs, c) for c in range(N_CORES)]
    res = run_bass_kernel_spmd(nc, in_maps, core_ids=list(range(N_CORES)))
    return host_combine(res.results, inputs)
